# revision 16
# baseline (speedup 1.0000x reference)
"""Trainium2 Bass kernel for nn_AttentionBlock (GroupNorm + 8-head self-attention + residual).

Full inputs in, full output out. Sharding: data-parallel over batch across the
8 NeuronCores (16 batches -> 2 per core), weights replicated, no collectives.

Layout strategy (per core, per batch; C=512 channels, S=1024 tokens):
  - x and xhat live as [C, S] tiles (channels on partitions) so GroupNorm
    scale/bias are per-partition scalars.
  - Q^T, K^T computed as [qk_rows, S] (head-major rows); V as [S, 8*65] with a
    ones-column per head (row-sums of exp fall out of the P@V matmul).
  - scores are computed TRANSPOSED: scoresT[j, i] = k_j . q_i so that the
    softmax reduction (over j) aligns with the matmul contraction axis and no
    transposes are ever needed.  exp() runs on ScalarE straight out of PSUM.
  - P@V gives resU^T [65, S] (row 64 = softmax denominators); normalization is
    reciprocal_approx_fast + gpsimd partition_broadcast + one DVE multiply.
  - out-projection consumes resT directly; residual-add fused in the epilogue.
All matmuls run as float32r (full PE rate at N=512, fp32 storage).
"""

import numpy as np

import concourse.bacc as bacc
import concourse.tile as tile
from concourse import mybir
from concourse.bass_utils import run_bass_kernel_spmd

N_CORES = 8
B, C, H, W = 16, 512, 32, 32
S = H * W                      # 1024
BL = B // N_CORES              # 2 batches per core
NH, DK = 8, 64
NG = 32                        # groupnorm groups
GSZ = C // NG                  # 16 channels per group
EPS = 1e-5
F32 = mybir.dt.float32
F32R = mybir.dt.float32r
AF = mybir.ActivationFunctionType
OP = mybir.AluOpType

# test.py can flip this to get a profiled run; results stashed in LAST.
TRACE = False
LAST = {}


def _build(has_bqk, has_bv, has_outb, debug=False):
    nc = bacc.Bacc()
    dbg = {}
    if debug:
        for nm, shp in (
            ("dbg_xh", [C, S]),
            ("dbg_qt", [C, S]),
            ("dbg_kt", [C, S]),
            ("dbg_v", [8, 128, NH * 65]),
            ("dbg_ex", [128, S]),
            ("dbg_pvt", [65, S]),
            ("dbg_rrow", [1, S]),
            ("dbg_rbt", [64, S]),
            ("dbg_rt", [C, S]),
        ):
            dbg[nm] = nc.dram_tensor(nm, shp, F32, kind="ExternalOutput")

    x_d = nc.dram_tensor("x", [BL, C, S], F32, kind="ExternalInput")
    wqt_d = nc.dram_tensor("wqt", [C, C], F32R, kind="ExternalInput")   # [c_in, q_row]
    wkt_d = nc.dram_tensor("wkt", [C, C], F32R, kind="ExternalInput")
    wvt_d = nc.dram_tensor("wvt", [C, C], F32R, kind="ExternalInput")
    wot_d = nc.dram_tensor("wot", [C, C], F32R, kind="ExternalInput")   # [d_out, c_out]
    g_d = nc.dram_tensor("gmat", [128, 8], F32, kind="ExternalInput")
    gt_d = nc.dram_tensor("gtmat", [8, 128], F32, kind="ExternalInput")
    vones_d = nc.dram_tensor("vones", [128, 8], F32R, kind="ExternalInput")
    bqk_d = (
        nc.dram_tensor("bqk", [128, 8], F32, kind="ExternalInput") if has_bqk else None
    )
    bv_d = nc.dram_tensor("bv", [1, C], F32R, kind="ExternalInput") if has_bv else None
    outb_d = (
        nc.dram_tensor("outb", [128, 4], F32, kind="ExternalInput") if has_outb else None
    )
    out_d = nc.dram_tensor("out", [BL, C, S], F32, kind="ExternalOutput")

    with tile.TileContext(nc) as tc:
        with (
            tc.tile_pool(name="const", bufs=1) as const,
            tc.tile_pool(name="px", bufs=8) as px,
            tc.tile_pool(name="pgn", bufs=4) as pgn,
            tc.tile_pool(name="pxh", bufs=4) as pxh,
            tc.tile_pool(name="pqt", bufs=4) as pqt,
            tc.tile_pool(name="pkt", bufs=4) as pkt,
            tc.tile_pool(name="pv", bufs=8) as pvp,
            tc.tile_pool(name="pexp", bufs=3) as pexp,
            tc.tile_pool(name="prec", bufs=2) as prec,
            tc.tile_pool(name="prt", bufs=4) as prt,
            tc.tile_pool(name="pout", bufs=3) as pout,
            tc.tile_pool(name="pps", bufs=2, space="PSUM") as pps,
            tc.tile_pool(name="psc", bufs=2, space="PSUM") as psc,
            tc.tile_pool(name="ppv", bufs=1, space="PSUM") as ppv,
        ):
            # ---- constants into SBUF
            wq_sb, wk_sb, wv_sb, wo_sb = [], [], [], []
            for nm, lst, src in (
                ("q", wq_sb, wqt_d),
                ("k", wk_sb, wkt_d),
                ("v", wv_sb, wvt_d),
                ("o", wo_sb, wot_d),
            ):
                for cb in range(4):
                    t = const.tile([128, C], F32R, tag=f"w_{nm}_{cb}")
                    nc.sync.dma_start(out=t, in_=src[cb * 128 : (cb + 1) * 128, :])
                    lst.append(t)
            g_sb = const.tile([128, 8], F32, tag="g")
            nc.sync.dma_start(out=g_sb, in_=g_d[:, :])
            gt_sb = const.tile([8, 128], F32, tag="gt")
            nc.sync.dma_start(out=gt_sb, in_=gt_d[:, :])
            eps_sb = const.tile([128, 1], F32, tag="eps")
            nc.vector.memset(eps_sb, EPS)
            if has_bqk:
                bqk_sb = const.tile([128, 8], F32, tag="bqk")
                nc.sync.dma_start(out=bqk_sb, in_=bqk_d[:, :])
            if has_bv:
                bv_sb = const.tile([1, C], F32R, tag="bv")
                nc.sync.dma_start(out=bv_sb, in_=bv_d[:, :])
                ones_sb = const.tile([1, S], F32R, tag="ones")
                nc.sync.dma_start(
                    out=ones_sb,
                    in_=vones_d[:, :].rearrange("(o p) f -> o (p f)", o=1),
                )
            if has_outb:
                outb_sb = const.tile([128, 4], F32, tag="outb")
                nc.sync.dma_start(out=outb_sb, in_=outb_d[:, :])

            for b in range(BL):
                # ---- load x [C, S] as 4 channel-block tiles (kept for residual)
                xt = []
                for cb in range(4):
                    t = px.tile([128, S], F32, tag="x")
                    nc.sync.dma_start(out=t, in_=x_d[b, cb * 128 : (cb + 1) * 128, :])
                    xt.append(t)

                # ---- groupnorm -> xhat (standardized; w/b folded into weights)
                xh = []
                for cb in range(4):
                    st6 = pgn.tile([128, 2, 6], F32, tag="st6")
                    nc.vector.bn_stats(out=st6[:, 0, :], in_=xt[cb][:, 0:512])
                    nc.vector.bn_stats(out=st6[:, 1, :], in_=xt[cb][:, 512:1024])
                    mv = pgn.tile([128, 2], F32, tag="mv")
                    nc.vector.bn_aggr(out=mv, in_=st6)
                    # me2 = [mean_c, E[x^2]_c]
                    me2 = pgn.tile([128, 2], F32, tag="me2")
                    nc.vector.tensor_copy(out=me2[:, 0:1], in_=mv[:, 0:1])
                    nc.vector.tensor_tensor(
                        out=me2[:, 1:2], in0=mv[:, 0:1], in1=mv[:, 0:1], op=OP.mult
                    )
                    nc.vector.tensor_tensor(
                        out=me2[:, 1:2], in0=me2[:, 1:2], in1=mv[:, 1:2], op=OP.add
                    )
                    # aggregate the 8 groups in this channel block (16 ch each)
                    pg = pps.tile([8, 2], F32, tag="pp")
                    nc.tensor.matmul(out=pg, lhsT=g_sb, rhs=me2, start=True, stop=True)
                    gs = pgn.tile([8, 2], F32, tag="gs")  # [gmean, grstd]
                    nc.scalar.mul(out=gs[:, 0:1], in_=pg[:, 0:1], mul=1.0 / GSZ)
                    tmp = pgn.tile([8, 2], F32, tag="tmp")
                    nc.vector.tensor_tensor(
                        out=tmp[:, 0:1], in0=gs[:, 0:1], in1=gs[:, 0:1], op=OP.mult
                    )
                    nc.vector.tensor_scalar(
                        out=tmp[:, 1:2],
                        in0=pg[:, 1:2],
                        scalar1=1.0 / GSZ,
                        scalar2=None,
                        op0=OP.mult,
                    )
                    nc.vector.tensor_tensor(
                        out=tmp[:, 1:2], in0=tmp[:, 1:2], in1=tmp[:, 0:1], op=OP.subtract
                    )
                    nc.scalar.activation(
                        out=gs[:, 1:2], in_=tmp[:, 1:2], func=AF.Sqrt, bias=eps_sb[0:8, :]
                    )
                    nc.vector.reciprocal(out=gs[:, 1:2], in_=gs[:, 1:2])
                    # broadcast group stats back to the 128 channels
                    pb = pps.tile([128, 2], F32, tag="pp")
                    nc.tensor.matmul(out=pb, lhsT=gt_sb, rhs=gs, start=True, stop=True)
                    t = pxh.tile([128, S], F32R, tag="xh")
                    nc.vector.tensor_scalar(
                        out=t,
                        in0=xt[cb],
                        scalar1=pb[:, 0:1],
                        scalar2=pb[:, 1:2],
                        op0=OP.subtract,
                        op1=OP.mult,
                    )
                    xh.append(t)
                if debug and b == 0:
                    for cb in range(4):
                        nc.sync.dma_start(
                            out=dbg["dbg_xh"][cb * 128 : (cb + 1) * 128, :],
                            in_=xh[cb].bitcast(F32),
                        )

                # ---- Q^T / K^T projections: [qk_row, S]
                qt, kt = [], []
                for dst, w_sb, boff in ((qt, wq_sb, 0), (kt, wk_sb, 4)):
                    for rb in range(4):
                        t = (pqt if dst is qt else pkt).tile([128, S], F32R, tag="qk")
                        for sc in range(2):
                            pq = pps.tile([128, 512], F32, tag="pp")
                            for cb in range(4):
                                nc.tensor.matmul(
                                    out=pq,
                                    lhsT=(w_sb[cb][:, rb * 128 : (rb + 1) * 128]),
                                    rhs=(xh[cb][:, sc * 512 : (sc + 1) * 512]),
                                    start=(cb == 0),
                                    stop=(cb == 3),
                                )
                            dst_ap = t[:, sc * 512 : (sc + 1) * 512]
                            if has_bqk:
                                nc.vector.tensor_scalar_add(
                                    out=dst_ap, in0=pq, scalar1=bqk_sb[:, boff + rb : boff + rb + 1]
                                )
                            else:
                                nc.vector.tensor_copy(out=dst_ap, in_=pq)
                        dst.append(t)
                if debug and b == 0:
                    for rb in range(4):
                        for dnm, lst2 in (("dbg_qt", qt), ("dbg_kt", kt)):
                            if len(lst2) == 4:
                                nc.sync.dma_start(
                                    out=dbg[dnm][rb * 128 : (rb + 1) * 128, :],
                                    in_=lst2[rb].bitcast(F32),
                                )

                # ---- V projection: [S, NH, 65] with ones column per head
                vt = []
                for st in range(8):
                    pv = pps.tile([128, 512], F32, tag="pp")
                    for cb in range(4):
                        nc.tensor.matmul(
                            out=pv,
                            lhsT=(xh[cb][:, st * 128 : (st + 1) * 128]),
                            rhs=(wv_sb[cb]),
                            start=(cb == 0),
                            stop=(cb == 3 and not has_bv),
                        )
                    if has_bv:
                        nc.tensor.matmul(
                            out=pv,
                            lhsT=(ones_sb[:, st * 128 : (st + 1) * 128]),
                            rhs=(bv_sb),
                            start=False,
                            stop=True,
                        )
                    t = pvp.tile([128, NH, 65], F32R, tag="v")
                    nc.sync.dma_start(
                        out=t[:, :, 64:65],
                        in_=vones_d[:, :].rearrange("p (h o) -> p h o", o=1),
                    )
                    nc.vector.tensor_copy(
                        out=t[:, :, 0:64], in_=pv.rearrange("p (h d) -> p h d", h=NH)
                    )
                    vt.append(t)
                if debug and b == 0:
                    for st in range(8):
                        if st < len(vt):
                            pass
                    if len(vt) == 8:
                        for st in range(8):
                            nc.sync.dma_start(
                                out=dbg["dbg_v"][st, :, :],
                                in_=vt[st].rearrange("p h d -> p (h d)").bitcast(F32),
                            )

                # ---- attention, head by head (scores transposed; sw-pipelined)
                rt = [
                    prt.tile([128, S], F32R, tag="rt", name=f"rt{i}") for i in range(4)
                ]
                for h in range(NH):
                    hp, off = h // 2, (h % 2) * 64
                    pvt = ppv.tile([65, S], F32, tag="ppvt")
                    ex_tiles = [None] * 8

                    def pv_matmuls(jb):
                        for sc in range(2):
                            nc.tensor.matmul(
                                out=pvt[:, sc * 512 : (sc + 1) * 512],
                                lhsT=(vt[jb][:, h, :]),
                                rhs=(ex_tiles[jb][:, sc * 512 : (sc + 1) * 512]),
                                start=(jb == 0),
                                stop=(jb == 7),
                            )

                    for jb in range(8):
                        ps = psc.tile([128, S], F32, tag="ps")
                        for sc in range(2):
                            nc.tensor.matmul(
                                out=ps[:, sc * 512 : (sc + 1) * 512],
                                lhsT=(kt[hp][off : off + 64, jb * 128 : (jb + 1) * 128]),
                                rhs=(qt[hp][off : off + 64, sc * 512 : (sc + 1) * 512]),
                                start=True,
                                stop=True,
                            )
                        ex = pexp.tile([128, S], F32R, tag="ex")
                        nc.scalar.activation(out=ex, in_=ps, func=AF.Exp)
                        ex_tiles[jb] = ex
                        if debug and b == 0 and h == 0 and jb == 0:
                            nc.sync.dma_start(out=dbg["dbg_ex"][:, :], in_=ex.bitcast(F32))
                        if jb >= 1:
                            pv_matmuls(jb - 1)
                    pv_matmuls(7)

                    if debug and b == 0 and h == 0:
                        dump = pout.tile([65, S], F32, tag="dump")
                        nc.vector.tensor_copy(out=dump, in_=pvt)
                        nc.sync.dma_start(out=dbg["dbg_pvt"][:, :], in_=dump)
                    # normalize: resT[head rows] = resU / rowsum
                    rrow = prec.tile([1, S], F32, tag="rr")
                    nc.vector.reciprocal(out=rrow, in_=pvt[64:65, :])
                    rbt = prec.tile([64, S], F32, tag="rb")
                    nc.gpsimd.partition_broadcast(rbt, rrow)
                    nc.vector.tensor_tensor(
                        out=rt[hp][off : off + 64, :], in0=pvt[0:64, :], in1=rbt, op=OP.mult
                    )
                    if debug and b == 0 and h == 0:
                        nc.sync.dma_start(out=dbg["dbg_rrow"][:, :], in_=rrow)
                        nc.sync.dma_start(out=dbg["dbg_rbt"][:, :], in_=rbt)

                if debug and b == 0:
                    for hp in range(4):
                        nc.sync.dma_start(
                            out=dbg["dbg_rt"][hp * 128 : (hp + 1) * 128, :],
                            in_=rt[hp].bitcast(F32),
                        )
                # ---- output projection + bias + residual, DMA out
                for cb in range(4):
                    ot = pout.tile([128, S], F32, tag="ot")
                    for sc in range(2):
                        po = pps.tile([128, 512], F32, tag="pp")
                        for db in range(4):
                            nc.tensor.matmul(
                                out=po,
                                lhsT=(wo_sb[db][:, cb * 128 : (cb + 1) * 128]),
                                rhs=(rt[db][:, sc * 512 : (sc + 1) * 512]),
                                start=(db == 0),
                                stop=(db == 3),
                            )
                        dst_ap = ot[:, sc * 512 : (sc + 1) * 512]
                        xsrc = xt[cb][:, sc * 512 : (sc + 1) * 512]
                        if has_outb:
                            nc.vector.scalar_tensor_tensor(
                                out=dst_ap,
                                in0=po,
                                scalar=outb_sb[:, cb : cb + 1],
                                in1=xsrc,
                                op0=OP.add,
                                op1=OP.add,
                            )
                        else:
                            nc.vector.tensor_tensor(
                                out=dst_ap, in0=po, in1=xsrc, op=OP.add
                            )
                    nc.sync.dma_start(
                        out=out_d[b, cb * 128 : (cb + 1) * 128, :], in_=ot
                    )

    nc.finalize()
    return nc


def kernel(**inputs):
    x = np.asarray(inputs["x"], np.float32)
    norm_w = np.asarray(inputs["norm_w"], np.float64)
    norm_b = np.asarray(inputs["norm_b"], np.float64)
    proj_w = np.asarray(inputs["proj_w"], np.float64)
    proj_b = np.asarray(inputs["proj_b"], np.float64)
    out_w = np.asarray(inputs["out_w"], np.float32)
    out_b = np.asarray(inputs["out_b"], np.float32)

    # split qkv rows (row = h*192 + t*64 + d, t in {q,k,v}) into head-major mats
    pw = proj_w.reshape(NH, 3, DK, C)
    pb = proj_b.reshape(NH, 3, DK)
    mats, biases = [], []
    for t in range(3):
        wm = pw[:, t].reshape(NH * DK, C)
        bv = pb[:, t].reshape(NH * DK)
        # fold groupnorm affine: y = xhat*nw + nb  =>  W@y + b = (W*nw)@xhat + (W@nb + b)
        mats.append(wm * norm_w[None, :])
        biases.append(bv + wm @ norm_b)
    wq, wk, wv = mats
    bq, bk, bv = biases
    scale = DK ** -0.5
    wq = wq * scale
    bq = bq * scale

    wqT = np.ascontiguousarray(wq.T, np.float32)
    wkT = np.ascontiguousarray(wk.T, np.float32)
    wvT = np.ascontiguousarray(wv.T, np.float32)
    woT = np.ascontiguousarray(out_w.T, np.float32)

    G = np.zeros((128, 8), np.float32)
    G[np.arange(128), np.arange(128) // GSZ] = 1.0
    GT = np.ascontiguousarray(G.T)
    VONES = np.ones((128, 8), np.float32)

    has_bqk = bool(np.any(bq) or np.any(bk))
    has_bv = bool(np.any(bv))
    has_outb = bool(np.any(out_b))

    bqk = np.zeros((128, 8), np.float32)
    bqk[:, 0:4] = bq.reshape(4, 128).T
    bqk[:, 4:8] = bk.reshape(4, 128).T
    outb128 = np.ascontiguousarray(out_b.reshape(4, 128).T)

    nc = _build(has_bqk, has_bv, has_outb)

    xr = x.reshape(B, C, S)
    in_maps = []
    for c in range(N_CORES):
        m = {
            "x": np.ascontiguousarray(xr[c * BL : (c + 1) * BL]),
            "wqt": wqT,
            "wkt": wkT,
            "wvt": wvT,
            "wot": woT,
            "gmat": G,
            "gtmat": GT,
            "vones": VONES,
        }
        if has_bqk:
            m["bqk"] = bqk
        if has_bv:
            m["bv"] = np.ascontiguousarray(bv.reshape(1, C), np.float32)
        if has_outb:
            m["outb"] = outb128
        in_maps.append(m)

    res = run_bass_kernel_spmd(
        nc, in_maps, core_ids=list(range(N_CORES)), trace=TRACE
    )
    LAST["exec_time_ns"] = res.exec_time_ns
    LAST["mean_exec_time_ns"] = res.mean_exec_time_ns
    LAST["result"] = res

    out = np.concatenate([res.results[c]["out"] for c in range(N_CORES)], axis=0)
    return np.ascontiguousarray(out.reshape(B, C, H, W).astype(np.float32))


# revision 17
# speedup vs baseline: 1.2506x; 1.2506x over previous
"""Trainium2 Bass kernel for nn_AttentionBlock (GroupNorm + 8-head self-attention + residual).

Full inputs in, full output out. Sharding: data-parallel over batch across the
8 NeuronCores (16 batches -> 2 per core), weights replicated, no collectives.

Layout strategy (per core, per batch; C=512 channels, S=1024 tokens):
  - x and xhat live as [C, S] tiles (channels on partitions) so GroupNorm
    scale/bias are per-partition scalars.  Cross-partition group reductions
    (16 channels/group) and the broadcast back are tiny PE matmuls against
    one-hot group matrices.
  - Q^T, K^T computed as [qk_rows, S] (head-major rows); V as [S, 8*65] with a
    ones-column per head (row-sums of exp fall out of the P@V matmul).
  - scores are computed TRANSPOSED: scoresT[j, i] = k_j . q_i so that the
    softmax reduction (over j) aligns with the matmul contraction axis and no
    transposes are ever needed.  exp() runs on ScalarE straight out of PSUM.
  - P@V gives resU^T [65, S] (row 64 = softmax denominators); normalization is
    a reciprocal of the sums row + gpsimd partition_broadcast + one DVE mul.
  - out-projection consumes resT directly; residual-add fused in the epilogue.
Attention/projection matmuls run in bf16 (fp32 PSUM accumulation); groupnorm
statistics stay fp32.  The softmax max-subtraction is skipped: scores are
~N(0,1) by construction (standardized activations, 1/sqrt(dk) folded into the
Q weights host-side), so exp() stays comfortably in fp32 range.
"""

import numpy as np
import ml_dtypes

import concourse.bacc as bacc
import concourse.tile as tile
from concourse import mybir
from concourse.bass_utils import run_bass_kernel_spmd

N_CORES = 8
B, C, H, W = 16, 512, 32, 32
S = H * W                      # 1024
BL = B // N_CORES              # 2 batches per core
NH, DK = 8, 64
NG = 32                        # groupnorm groups
GSZ = C // NG                  # 16 channels per group
EPS = 1e-5
F32 = mybir.dt.float32
BF16 = mybir.dt.bfloat16
AF = mybir.ActivationFunctionType
OP = mybir.AluOpType
NPBF16 = ml_dtypes.bfloat16

# test.py can flip these; results stashed in LAST.
TRACE = False
RECIP_MODE = "approx_sbuf"  # "approx_sbuf" | "plain"
LAST = {}


def _build(has_bqk, has_bv, has_outb, debug=False):
    nc = bacc.Bacc()
    dbg = {}
    if debug:
        for nm, shp in (
            ("dbg_xh", [C, S]),
            ("dbg_qt", [C, S]),
            ("dbg_kt", [C, S]),
            ("dbg_v", [8, 128, NH * 65]),
            ("dbg_ex", [128, S]),
            ("dbg_pvt", [65, S]),
            ("dbg_rrow", [1, S]),
            ("dbg_rbt", [64, S]),
            ("dbg_rt", [C, S]),
        ):
            dbg[nm] = nc.dram_tensor(nm, shp, F32, kind="ExternalOutput")

    x_d = nc.dram_tensor("x", [BL, C, S], F32, kind="ExternalInput")
    wqt_d = nc.dram_tensor("wqt", [C, C], BF16, kind="ExternalInput")   # [c_in, q_row]
    wkt_d = nc.dram_tensor("wkt", [C, C], BF16, kind="ExternalInput")
    wvt_d = nc.dram_tensor("wvt", [C, C], BF16, kind="ExternalInput")
    wot_d = nc.dram_tensor("wot", [C, C], BF16, kind="ExternalInput")   # [d_out, c_out]
    g_d = nc.dram_tensor("gmat", [128, 8], F32, kind="ExternalInput")
    gt_d = nc.dram_tensor("gtmat", [8, 128], F32, kind="ExternalInput")
    bqk_d = (
        nc.dram_tensor("bqk", [128, 8], F32, kind="ExternalInput") if has_bqk else None
    )
    bv_d = nc.dram_tensor("bv", [1, C], BF16, kind="ExternalInput") if has_bv else None
    outb_d = (
        nc.dram_tensor("outb", [128, 4], F32, kind="ExternalInput") if has_outb else None
    )
    out_d = nc.dram_tensor("out", [BL, C, S], F32, kind="ExternalOutput")

    with tile.TileContext(nc) as tc:
        with (
            tc.tile_pool(name="const", bufs=1) as const,
            tc.tile_pool(name="px", bufs=8) as px,
            tc.tile_pool(name="pgn", bufs=4) as pgn,
            tc.tile_pool(name="pxh", bufs=8) as pxh,
            tc.tile_pool(name="pqt", bufs=8) as pqt,
            tc.tile_pool(name="pkt", bufs=8) as pkt,
            tc.tile_pool(name="pv", bufs=16) as pvp,
            tc.tile_pool(name="pexp", bufs=3) as pexp,
            tc.tile_pool(name="prec", bufs=2) as prec,
            tc.tile_pool(name="prt", bufs=8) as prt,
            tc.tile_pool(name="pout", bufs=3) as pout,
            tc.tile_pool(name="pps", bufs=2, space="PSUM") as pps,
            tc.tile_pool(name="psc", bufs=2, space="PSUM") as psc,
            tc.tile_pool(name="ppv", bufs=1, space="PSUM") as ppv,
        ):
            # ---- constants into SBUF
            wq_sb, wk_sb, wv_sb, wo_sb = [], [], [], []
            for nm, lst, src in (
                ("q", wq_sb, wqt_d),
                ("k", wk_sb, wkt_d),
                ("v", wv_sb, wvt_d),
                ("o", wo_sb, wot_d),
            ):
                for cb in range(4):
                    t = const.tile([128, C], BF16, tag=f"w_{nm}_{cb}")
                    nc.sync.dma_start(out=t, in_=src[cb * 128 : (cb + 1) * 128, :])
                    lst.append(t)
            g_sb = const.tile([128, 8], F32, tag="g")
            nc.sync.dma_start(out=g_sb, in_=g_d[:, :])
            gt_sb = const.tile([8, 128], F32, tag="gt")
            nc.sync.dma_start(out=gt_sb, in_=gt_d[:, :])
            eps_sb = const.tile([128, 1], F32, tag="eps")
            nc.vector.memset(eps_sb, EPS)
            if has_bqk:
                bqk_sb = const.tile([128, 8], F32, tag="bqk")
                nc.sync.dma_start(out=bqk_sb, in_=bqk_d[:, :])
            if has_bv:
                bv_sb = const.tile([1, C], BF16, tag="bv")
                nc.sync.dma_start(out=bv_sb, in_=bv_d[:, :])
                ones_sb = const.tile([1, S], BF16, tag="ones")
                nc.vector.memset(ones_sb, 1.0)
            if has_outb:
                outb_sb = const.tile([128, 4], F32, tag="outb")
                nc.sync.dma_start(out=outb_sb, in_=outb_d[:, :])

            for b in range(BL):
                # ---- load x [C, S] as 4 channel-block tiles (kept for residual)
                xt = []
                for cb in range(4):
                    t = px.tile([128, S], F32, tag="x")
                    nc.sync.dma_start(out=t, in_=x_d[b, cb * 128 : (cb + 1) * 128, :])
                    xt.append(t)

                # ---- groupnorm -> xhat (standardized; w/b folded into weights)
                xh = []
                for cb in range(4):
                    st6 = pgn.tile([128, 2, 6], F32, tag="st6")
                    nc.vector.bn_stats(out=st6[:, 0, :], in_=xt[cb][:, 0:512])
                    nc.vector.bn_stats(out=st6[:, 1, :], in_=xt[cb][:, 512:1024])
                    mv = pgn.tile([128, 2], F32, tag="mv")
                    nc.vector.bn_aggr(out=mv, in_=st6)
                    # me2 = [mean_c, E[x^2]_c]
                    me2 = pgn.tile([128, 2], F32, tag="me2")
                    nc.vector.tensor_copy(out=me2[:, 0:1], in_=mv[:, 0:1])
                    nc.vector.tensor_tensor(
                        out=me2[:, 1:2], in0=mv[:, 0:1], in1=mv[:, 0:1], op=OP.mult
                    )
                    nc.vector.tensor_tensor(
                        out=me2[:, 1:2], in0=me2[:, 1:2], in1=mv[:, 1:2], op=OP.add
                    )
                    # aggregate the 8 groups in this channel block (16 ch each)
                    pg = pps.tile([8, 2], F32, tag="pp")
                    nc.tensor.matmul(out=pg, lhsT=g_sb, rhs=me2, start=True, stop=True)
                    gs = pgn.tile([8, 2], F32, tag="gs")  # [gmean, grstd]
                    nc.scalar.mul(out=gs[:, 0:1], in_=pg[:, 0:1], mul=1.0 / GSZ)
                    tmp = pgn.tile([8, 2], F32, tag="tmp")
                    nc.vector.tensor_tensor(
                        out=tmp[:, 0:1], in0=gs[:, 0:1], in1=gs[:, 0:1], op=OP.mult
                    )
                    nc.vector.tensor_scalar(
                        out=tmp[:, 1:2],
                        in0=pg[:, 1:2],
                        scalar1=1.0 / GSZ,
                        scalar2=None,
                        op0=OP.mult,
                    )
                    nc.vector.tensor_tensor(
                        out=tmp[:, 1:2], in0=tmp[:, 1:2], in1=tmp[:, 0:1], op=OP.subtract
                    )
                    nc.scalar.activation(
                        out=gs[:, 1:2], in_=tmp[:, 1:2], func=AF.Sqrt, bias=eps_sb[0:8, :]
                    )
                    nc.vector.reciprocal(out=gs[:, 1:2], in_=gs[:, 1:2])
                    # broadcast group stats back to the 128 channels
                    pb = pps.tile([128, 2], F32, tag="pp")
                    nc.tensor.matmul(out=pb, lhsT=gt_sb, rhs=gs, start=True, stop=True)
                    t = pxh.tile([128, S], BF16, tag="xh")
                    nc.vector.tensor_scalar(
                        out=t,
                        in0=xt[cb],
                        scalar1=pb[:, 0:1],
                        scalar2=pb[:, 1:2],
                        op0=OP.subtract,
                        op1=OP.mult,
                    )
                    xh.append(t)
                if debug and b == 0:
                    for cb in range(4):
                        nc.sync.dma_start(
                            out=dbg["dbg_xh"][cb * 128 : (cb + 1) * 128, :],
                            in_=xh[cb],
                        )

                # ---- Q^T / K^T projections: [qk_row, S]
                qt, kt = [], []
                for dst, w_sb, boff in ((qt, wq_sb, 0), (kt, wk_sb, 4)):
                    for rb in range(4):
                        t = (pqt if dst is qt else pkt).tile([128, S], BF16, tag="qk")
                        for sc in range(2):
                            pq = pps.tile([128, 512], F32, tag="pp")
                            for cb in range(4):
                                nc.tensor.matmul(
                                    out=pq,
                                    lhsT=w_sb[cb][:, rb * 128 : (rb + 1) * 128],
                                    rhs=xh[cb][:, sc * 512 : (sc + 1) * 512],
                                    start=(cb == 0),
                                    stop=(cb == 3),
                                )
                            dst_ap = t[:, sc * 512 : (sc + 1) * 512]
                            if has_bqk:
                                nc.vector.tensor_scalar_add(
                                    out=dst_ap,
                                    in0=pq,
                                    scalar1=bqk_sb[:, boff + rb : boff + rb + 1],
                                )
                            else:
                                nc.vector.tensor_copy(out=dst_ap, in_=pq)
                        dst.append(t)
                if debug and b == 0:
                    for rb in range(4):
                        for dnm, lst2 in (("dbg_qt", qt), ("dbg_kt", kt)):
                            nc.sync.dma_start(
                                out=dbg[dnm][rb * 128 : (rb + 1) * 128, :],
                                in_=lst2[rb],
                            )

                # ---- V projection: [S, NH, 65] with ones column per head
                vt = []
                for st in range(8):
                    pv = pps.tile([128, 512], F32, tag="pp")
                    for cb in range(4):
                        nc.tensor.matmul(
                            out=pv,
                            lhsT=xh[cb][:, st * 128 : (st + 1) * 128],
                            rhs=wv_sb[cb],
                            start=(cb == 0),
                            stop=(cb == 3 and not has_bv),
                        )
                    if has_bv:
                        nc.tensor.matmul(
                            out=pv,
                            lhsT=ones_sb[:, st * 128 : (st + 1) * 128],
                            rhs=bv_sb,
                            start=False,
                            stop=True,
                        )
                    t = pvp.tile([128, NH, 65], BF16, tag="v")
                    nc.vector.memset(t[:, :, 64:65], 1.0)
                    nc.vector.tensor_copy(
                        out=t[:, :, 0:64], in_=pv.rearrange("p (h d) -> p h d", h=NH)
                    )
                    vt.append(t)
                if debug and b == 0:
                    for st in range(8):
                        nc.sync.dma_start(
                            out=dbg["dbg_v"][st, :, :],
                            in_=vt[st].rearrange("p h d -> p (h d)"),
                        )

                # ---- attention, head by head (scores transposed; sw-pipelined)
                rt = [
                    prt.tile([128, S], BF16, tag="rt", name=f"rt{i}") for i in range(4)
                ]
                for h in range(NH):
                    hp, off = h // 2, (h % 2) * 64
                    pvt = ppv.tile([65, S], F32, tag="ppvt")
                    ex_tiles = [None] * 8

                    def pv_matmuls(jb):
                        for sc in range(2):
                            nc.tensor.matmul(
                                out=pvt[:, sc * 512 : (sc + 1) * 512],
                                lhsT=vt[jb][:, h, :],
                                rhs=ex_tiles[jb][:, sc * 512 : (sc + 1) * 512],
                                start=(jb == 0),
                                stop=(jb == 7),
                            )

                    for jb in range(8):
                        ps = psc.tile([128, S], F32, tag="ps")
                        for sc in range(2):
                            nc.tensor.matmul(
                                out=ps[:, sc * 512 : (sc + 1) * 512],
                                lhsT=kt[hp][off : off + 64, jb * 128 : (jb + 1) * 128],
                                rhs=qt[hp][off : off + 64, sc * 512 : (sc + 1) * 512],
                                start=True,
                                stop=True,
                            )
                        ex = pexp.tile([128, S], BF16, tag="ex")
                        nc.scalar.activation(out=ex, in_=ps, func=AF.Exp)
                        ex_tiles[jb] = ex
                        if debug and b == 0 and h == 0 and jb == 0:
                            nc.sync.dma_start(out=dbg["dbg_ex"][:, :], in_=ex)
                        if jb >= 1:
                            pv_matmuls(jb - 1)
                    pv_matmuls(7)

                    if debug and b == 0 and h == 0:
                        dump = pout.tile([65, S], F32, tag="dump")
                        nc.vector.tensor_copy(out=dump, in_=pvt)
                        nc.sync.dma_start(out=dbg["dbg_pvt"][:, :], in_=dump)
                    # normalize: resT[head rows] = resU / rowsum
                    rrow = prec.tile([1, S], F32, tag="rr")
                    if RECIP_MODE == "approx_sbuf":
                        stage = prec.tile([1, S], F32, tag="st")
                        nc.vector.tensor_copy(out=stage, in_=pvt[64:65, :])
                        nc.vector.reciprocal_approx_fast(out=rrow, in_=stage)
                    else:
                        nc.vector.reciprocal(out=rrow, in_=pvt[64:65, :])
                    rbt = prec.tile([64, S], F32, tag="rb")
                    nc.gpsimd.partition_broadcast(rbt, rrow)
                    nc.vector.tensor_tensor(
                        out=rt[hp][off : off + 64, :], in0=pvt[0:64, :], in1=rbt, op=OP.mult
                    )
                    if debug and b == 0 and h == 0:
                        nc.sync.dma_start(out=dbg["dbg_rrow"][:, :], in_=rrow)
                        nc.sync.dma_start(out=dbg["dbg_rbt"][:, :], in_=rbt)

                if debug and b == 0:
                    for hp in range(4):
                        nc.sync.dma_start(
                            out=dbg["dbg_rt"][hp * 128 : (hp + 1) * 128, :],
                            in_=rt[hp],
                        )
                # ---- output projection + bias + residual, DMA out
                for cb in range(4):
                    ot = pout.tile([128, S], F32, tag="ot")
                    for sc in range(2):
                        po = pps.tile([128, 512], F32, tag="pp")
                        for db in range(4):
                            nc.tensor.matmul(
                                out=po,
                                lhsT=wo_sb[db][:, cb * 128 : (cb + 1) * 128],
                                rhs=rt[db][:, sc * 512 : (sc + 1) * 512],
                                start=(db == 0),
                                stop=(db == 3),
                            )
                        dst_ap = ot[:, sc * 512 : (sc + 1) * 512]
                        xsrc = xt[cb][:, sc * 512 : (sc + 1) * 512]
                        if has_outb:
                            nc.vector.scalar_tensor_tensor(
                                out=dst_ap,
                                in0=po,
                                scalar=outb_sb[:, cb : cb + 1],
                                in1=xsrc,
                                op0=OP.add,
                                op1=OP.add,
                            )
                        else:
                            nc.vector.tensor_tensor(
                                out=dst_ap, in0=po, in1=xsrc, op=OP.add
                            )
                    nc.sync.dma_start(
                        out=out_d[b, cb * 128 : (cb + 1) * 128, :], in_=ot
                    )

    nc.finalize()
    return nc


def kernel(**inputs):
    x = np.asarray(inputs["x"], np.float32)
    norm_w = np.asarray(inputs["norm_w"], np.float64)
    norm_b = np.asarray(inputs["norm_b"], np.float64)
    proj_w = np.asarray(inputs["proj_w"], np.float64)
    proj_b = np.asarray(inputs["proj_b"], np.float64)
    out_w = np.asarray(inputs["out_w"], np.float32)
    out_b = np.asarray(inputs["out_b"], np.float32)

    # split qkv rows (row = h*192 + t*64 + d, t in {q,k,v}) into head-major mats
    pw = proj_w.reshape(NH, 3, DK, C)
    pb = proj_b.reshape(NH, 3, DK)
    mats, biases = [], []
    for t in range(3):
        wm = pw[:, t].reshape(NH * DK, C)
        bv = pb[:, t].reshape(NH * DK)
        # fold groupnorm affine: y = xhat*nw + nb  =>  W@y + b = (W*nw)@xhat + (W@nb + b)
        mats.append(wm * norm_w[None, :])
        biases.append(bv + wm @ norm_b)
    wq, wk, wv = mats
    bq, bk, bv = biases
    scale = DK ** -0.5
    wq = wq * scale
    bq = bq * scale

    wqT = np.ascontiguousarray(wq.T).astype(NPBF16)
    wkT = np.ascontiguousarray(wk.T).astype(NPBF16)
    wvT = np.ascontiguousarray(wv.T).astype(NPBF16)
    woT = np.ascontiguousarray(out_w.T).astype(NPBF16)

    G = np.zeros((128, 8), np.float32)
    G[np.arange(128), np.arange(128) // GSZ] = 1.0
    GT = np.ascontiguousarray(G.T)

    has_bqk = bool(np.any(bq) or np.any(bk))
    has_bv = bool(np.any(bv))
    has_outb = bool(np.any(out_b))

    bqk = np.zeros((128, 8), np.float32)
    bqk[:, 0:4] = bq.reshape(4, 128).T
    bqk[:, 4:8] = bk.reshape(4, 128).T
    outb128 = np.ascontiguousarray(out_b.reshape(4, 128).T)

    nc = _build(has_bqk, has_bv, has_outb)

    xr = x.reshape(B, C, S)
    in_maps = []
    for c in range(N_CORES):
        m = {
            "x": np.ascontiguousarray(xr[c * BL : (c + 1) * BL]),
            "wqt": wqT,
            "wkt": wkT,
            "wvt": wvT,
            "wot": woT,
            "gmat": G,
            "gtmat": GT,
        }
        if has_bqk:
            m["bqk"] = bqk
        if has_bv:
            m["bv"] = np.ascontiguousarray(bv.reshape(1, C)).astype(NPBF16)
        if has_outb:
            m["outb"] = outb128
        in_maps.append(m)

    res = run_bass_kernel_spmd(
        nc, in_maps, core_ids=list(range(N_CORES)), trace=TRACE
    )
    LAST["exec_time_ns"] = res.exec_time_ns
    LAST["mean_exec_time_ns"] = res.mean_exec_time_ns
    LAST["result"] = res

    out = np.concatenate([res.results[c]["out"] for c in range(N_CORES)], axis=0)
    return np.ascontiguousarray(out.reshape(B, C, H, W).astype(np.float32))


# revision 19
# speedup vs baseline: 1.5314x; 1.2245x over previous
"""Trainium2 Bass kernel for nn_AttentionBlock (GroupNorm + 8-head self-attention + residual).

Full inputs in, full output out. Sharding: data-parallel over batch across the
8 NeuronCores (16 batches -> 2 per core), weights replicated, no collectives.

Layout strategy (per core, per batch; C=512 channels, S=1024 tokens):
  - x and xhat live as [C, S] tiles (channels on partitions) so GroupNorm
    scale/bias are per-partition scalars.  Cross-partition group reductions
    (16 channels/group) and the broadcast back are tiny PE matmuls against
    one-hot group matrices.
  - Q^T, K^T computed as [qk_rows, S] (head-major rows); V as [S, 8*65] with a
    ones-column per head (row-sums of exp fall out of the P@V matmul).
  - scores are computed TRANSPOSED: scoresT[j, i] = k_j . q_i so that the
    softmax reduction (over j) aligns with the matmul contraction axis and no
    transposes are ever needed.  exp() runs on ScalarE straight out of PSUM.
  - P@V gives resU^T [65, S] (row 64 = softmax denominators); normalization is
    a reciprocal of the sums row + gpsimd partition_broadcast + one DVE mul.
  - out-projection consumes resT directly; residual-add fused in the epilogue.
Attention/projection matmuls run in bf16 (fp32 PSUM accumulation); groupnorm
statistics stay fp32.  The softmax max-subtraction is skipped: scores are
~N(0,1) by construction (standardized activations, 1/sqrt(dk) folded into the
Q weights host-side), so exp() stays comfortably in fp32 range.
"""

import numpy as np
import ml_dtypes

import concourse.bacc as bacc
import concourse.tile as tile
from concourse import mybir
from concourse.bass_utils import run_bass_kernel_spmd

N_CORES = 8
B, C, H, W = 16, 512, 32, 32
S = H * W                      # 1024
BL = B // N_CORES              # 2 batches per core
NH, DK = 8, 64
NG = 32                        # groupnorm groups
GSZ = C // NG                  # 16 channels per group
EPS = 1e-5
F32 = mybir.dt.float32
BF16 = mybir.dt.bfloat16
AF = mybir.ActivationFunctionType
OP = mybir.AluOpType
NPBF16 = ml_dtypes.bfloat16

# test.py can flip these; results stashed in LAST.
TRACE = False
RECIP_MODE = "approx_sbuf"  # "approx_sbuf" | "plain"
LAST = {}


def _build(has_bqk, has_bv, has_outb, debug=False):
    nc = bacc.Bacc()
    dbg = {}
    if debug:
        for nm, shp in (
            ("dbg_xh", [C, S]),
            ("dbg_qt", [C, S]),
            ("dbg_kt", [C, S]),
            ("dbg_v", [8, 128, NH * 65]),
            ("dbg_ex", [128, S]),
            ("dbg_pvt", [65, S]),
            ("dbg_rrow", [1, S]),
            ("dbg_rbt", [64, S]),
            ("dbg_rt", [C, S]),
        ):
            dbg[nm] = nc.dram_tensor(nm, shp, F32, kind="ExternalOutput")

    x_d = nc.dram_tensor("x", [BL, C, S], F32, kind="ExternalInput")
    wqt_d = nc.dram_tensor("wqt", [C, C], BF16, kind="ExternalInput")   # [c_in, q_row]
    wkt_d = nc.dram_tensor("wkt", [C, C], BF16, kind="ExternalInput")
    wvt_d = nc.dram_tensor("wvt", [C, C], BF16, kind="ExternalInput")
    wot_d = nc.dram_tensor("wot", [C, C], BF16, kind="ExternalInput")   # [d_out, c_out]
    g_d = nc.dram_tensor("gmat", [128, 8], F32, kind="ExternalInput")
    gt_d = nc.dram_tensor("gtmat", [8, 128], F32, kind="ExternalInput")
    bqk_d = (
        nc.dram_tensor("bqk", [128, 8], F32, kind="ExternalInput") if has_bqk else None
    )
    bv_d = nc.dram_tensor("bv", [1, C], BF16, kind="ExternalInput") if has_bv else None
    outb_d = (
        nc.dram_tensor("outb", [128, 4], F32, kind="ExternalInput") if has_outb else None
    )
    out_d = nc.dram_tensor("out", [BL, C, S], F32, kind="ExternalOutput")

    with tile.TileContext(nc) as tc:
        with (
            tc.tile_pool(name="const", bufs=1) as const,
            tc.tile_pool(name="px", bufs=8) as px,
            tc.tile_pool(name="pgn", bufs=4) as pgn,
            tc.tile_pool(name="pxh", bufs=8) as pxh,
            tc.tile_pool(name="pqt", bufs=8) as pqt,
            tc.tile_pool(name="pkt", bufs=8) as pkt,
            tc.tile_pool(name="pv", bufs=16) as pvp,
            tc.tile_pool(name="pexp", bufs=10) as pexp,
            tc.tile_pool(name="prec", bufs=3) as prec,
            tc.tile_pool(name="prt", bufs=8) as prt,
            tc.tile_pool(name="pout", bufs=3) as pout,
            tc.tile_pool(name="pps", bufs=2, space="PSUM") as pps,
            tc.tile_pool(name="psc", bufs=2, space="PSUM") as psc,
            tc.tile_pool(name="ppv", bufs=2, space="PSUM") as ppv,
        ):
            # ---- constants into SBUF
            wq_sb, wk_sb, wv_sb, wo_sb = [], [], [], []
            for nm, lst, src in (
                ("q", wq_sb, wqt_d),
                ("k", wk_sb, wkt_d),
                ("v", wv_sb, wvt_d),
                ("o", wo_sb, wot_d),
            ):
                for cb in range(4):
                    t = const.tile([128, C], BF16, tag=f"w_{nm}_{cb}")
                    nc.sync.dma_start(out=t, in_=src[cb * 128 : (cb + 1) * 128, :])
                    lst.append(t)
            g_sb = const.tile([128, 8], F32, tag="g")
            nc.sync.dma_start(out=g_sb, in_=g_d[:, :])
            gt_sb = const.tile([8, 128], F32, tag="gt")
            nc.sync.dma_start(out=gt_sb, in_=gt_d[:, :])
            eps_sb = const.tile([128, 1], F32, tag="eps")
            nc.vector.memset(eps_sb, EPS)
            if has_bqk:
                bqk_sb = const.tile([128, 8], F32, tag="bqk")
                nc.sync.dma_start(out=bqk_sb, in_=bqk_d[:, :])
            if has_bv:
                bv_sb = const.tile([1, C], BF16, tag="bv")
                nc.sync.dma_start(out=bv_sb, in_=bv_d[:, :])
                ones_sb = const.tile([1, S], BF16, tag="ones")
                nc.vector.memset(ones_sb, 1.0)
            if has_outb:
                outb_sb = const.tile([128, 4], F32, tag="outb")
                nc.sync.dma_start(out=outb_sb, in_=outb_d[:, :])

            for b in range(BL):
                # ---- load x [C, S] as 4 channel-block tiles (kept for residual)
                xt = []
                for cb in range(4):
                    t = px.tile([128, S], F32, tag="x")
                    nc.sync.dma_start(out=t, in_=x_d[b, cb * 128 : (cb + 1) * 128, :])
                    xt.append(t)

                # ---- groupnorm -> xhat (standardized; w/b folded into weights)
                xh = []
                for cb in range(4):
                    st6 = pgn.tile([128, 2, 6], F32, tag="st6")
                    nc.vector.bn_stats(out=st6[:, 0, :], in_=xt[cb][:, 0:512])
                    nc.vector.bn_stats(out=st6[:, 1, :], in_=xt[cb][:, 512:1024])
                    mv = pgn.tile([128, 2], F32, tag="mv")
                    nc.vector.bn_aggr(out=mv, in_=st6)
                    # me2 = [mean_c, E[x^2]_c]
                    me2 = pgn.tile([128, 2], F32, tag="me2")
                    nc.vector.tensor_copy(out=me2[:, 0:1], in_=mv[:, 0:1])
                    nc.vector.tensor_tensor(
                        out=me2[:, 1:2], in0=mv[:, 0:1], in1=mv[:, 0:1], op=OP.mult
                    )
                    nc.vector.tensor_tensor(
                        out=me2[:, 1:2], in0=me2[:, 1:2], in1=mv[:, 1:2], op=OP.add
                    )
                    # aggregate the 8 groups in this channel block (16 ch each)
                    pg = pps.tile([8, 2], F32, tag="pp")
                    nc.tensor.matmul(out=pg, lhsT=g_sb, rhs=me2, start=True, stop=True)
                    gs = pgn.tile([8, 2], F32, tag="gs")  # [gmean, grstd]
                    nc.scalar.mul(out=gs[:, 0:1], in_=pg[:, 0:1], mul=1.0 / GSZ)
                    tmp = pgn.tile([8, 2], F32, tag="tmp")
                    nc.vector.tensor_tensor(
                        out=tmp[:, 0:1], in0=gs[:, 0:1], in1=gs[:, 0:1], op=OP.mult
                    )
                    nc.vector.tensor_scalar(
                        out=tmp[:, 1:2],
                        in0=pg[:, 1:2],
                        scalar1=1.0 / GSZ,
                        scalar2=None,
                        op0=OP.mult,
                    )
                    nc.vector.tensor_tensor(
                        out=tmp[:, 1:2], in0=tmp[:, 1:2], in1=tmp[:, 0:1], op=OP.subtract
                    )
                    nc.scalar.activation(
                        out=gs[:, 1:2], in_=tmp[:, 1:2], func=AF.Sqrt, bias=eps_sb[0:8, :]
                    )
                    nc.vector.reciprocal(out=gs[:, 1:2], in_=gs[:, 1:2])
                    # broadcast group stats back to the 128 channels
                    pb = pps.tile([128, 2], F32, tag="pp")
                    nc.tensor.matmul(out=pb, lhsT=gt_sb, rhs=gs, start=True, stop=True)
                    t = pxh.tile([128, S], BF16, tag="xh")
                    nc.vector.tensor_scalar(
                        out=t,
                        in0=xt[cb],
                        scalar1=pb[:, 0:1],
                        scalar2=pb[:, 1:2],
                        op0=OP.subtract,
                        op1=OP.mult,
                    )
                    xh.append(t)
                if debug and b == 0:
                    for cb in range(4):
                        nc.sync.dma_start(
                            out=dbg["dbg_xh"][cb * 128 : (cb + 1) * 128, :],
                            in_=xh[cb],
                        )

                # ---- Q^T / K^T projections: [qk_row, S]
                qt, kt = [], []
                for dst, w_sb, boff in ((qt, wq_sb, 0), (kt, wk_sb, 4)):
                    for rb in range(4):
                        t = (pqt if dst is qt else pkt).tile([128, S], BF16, tag="qk")
                        for sc in range(2):
                            pq = pps.tile([128, 512], F32, tag="pp")
                            for cb in range(4):
                                nc.tensor.matmul(
                                    out=pq,
                                    lhsT=w_sb[cb][:, rb * 128 : (rb + 1) * 128],
                                    rhs=xh[cb][:, sc * 512 : (sc + 1) * 512],
                                    start=(cb == 0),
                                    stop=(cb == 3),
                                )
                            dst_ap = t[:, sc * 512 : (sc + 1) * 512]
                            if has_bqk:
                                nc.vector.tensor_scalar_add(
                                    out=dst_ap,
                                    in0=pq,
                                    scalar1=bqk_sb[:, boff + rb : boff + rb + 1],
                                )
                            else:
                                nc.vector.tensor_copy(out=dst_ap, in_=pq)
                        dst.append(t)
                if debug and b == 0:
                    for rb in range(4):
                        for dnm, lst2 in (("dbg_qt", qt), ("dbg_kt", kt)):
                            nc.sync.dma_start(
                                out=dbg[dnm][rb * 128 : (rb + 1) * 128, :],
                                in_=lst2[rb],
                            )

                # ---- V projection: [S, NH, 65] with ones column per head
                vt = []
                for st in range(8):
                    pv = pps.tile([128, 512], F32, tag="pp")
                    for cb in range(4):
                        nc.tensor.matmul(
                            out=pv,
                            lhsT=xh[cb][:, st * 128 : (st + 1) * 128],
                            rhs=wv_sb[cb],
                            start=(cb == 0),
                            stop=(cb == 3 and not has_bv),
                        )
                    if has_bv:
                        nc.tensor.matmul(
                            out=pv,
                            lhsT=ones_sb[:, st * 128 : (st + 1) * 128],
                            rhs=bv_sb,
                            start=False,
                            stop=True,
                        )
                    t = pvp.tile([128, NH, 65], BF16, tag="v")
                    nc.vector.memset(t[:, :, 64:65], 1.0)
                    nc.vector.tensor_copy(
                        out=t[:, :, 0:64], in_=pv.rearrange("p (h d) -> p h d", h=NH)
                    )
                    vt.append(t)
                if debug and b == 0:
                    for st in range(8):
                        nc.sync.dma_start(
                            out=dbg["dbg_v"][st, :, :],
                            in_=vt[st].rearrange("p h d -> p (h d)"),
                        )

                # ---- attention, head by head (scores transposed; sw-pipelined)
                rt = [
                    prt.tile([128, S], BF16, tag="rt", name=f"rt{i}") for i in range(4)
                ]
                for h in range(NH):
                    hp, off = h // 2, (h % 2) * 64
                    ex_tiles = [None] * 8

                    for jb in range(8):
                        ps = psc.tile([128, S], F32, tag="ps")
                        for sc in range(2):
                            nc.tensor.matmul(
                                out=ps[:, sc * 512 : (sc + 1) * 512],
                                lhsT=kt[hp][off : off + 64, jb * 128 : (jb + 1) * 128],
                                rhs=qt[hp][off : off + 64, sc * 512 : (sc + 1) * 512],
                                start=True,
                                stop=True,
                            )
                        ex = pexp.tile([128, S], BF16, tag="ex")
                        nc.scalar.activation(out=ex, in_=ps, func=AF.Exp)
                        ex_tiles[jb] = ex
                        if debug and b == 0 and h == 0 and jb == 0:
                            nc.sync.dma_start(out=dbg["dbg_ex"][:, :], in_=ex)

                    # P@V per 512-column half into a single-bank accumulator;
                    # each half's normalize overlaps the other half's matmuls.
                    for sc in range(2):
                        pvt = ppv.tile([65, 512], F32, tag="ppvt")
                        for jb in range(8):
                            nc.tensor.matmul(
                                out=pvt,
                                lhsT=vt[jb][:, h, :],
                                rhs=ex_tiles[jb][:, sc * 512 : (sc + 1) * 512],
                                start=(jb == 0),
                                stop=(jb == 7),
                            )
                        if debug and b == 0 and h == 0:
                            dump = pout.tile([65, 512], F32, tag="dump", name=f"dmp{sc}")
                            nc.vector.tensor_copy(out=dump, in_=pvt)
                            nc.sync.dma_start(
                                out=dbg["dbg_pvt"][:, sc * 512 : (sc + 1) * 512],
                                in_=dump,
                            )
                        # normalize: resT[head rows] = resU / rowsum
                        rrow = prec.tile([1, 512], F32, tag="rr")
                        if RECIP_MODE == "approx_sbuf":
                            stage = prec.tile([1, 512], F32, tag="st")
                            nc.vector.tensor_copy(out=stage, in_=pvt[64:65, :])
                            nc.vector.reciprocal_approx_fast(out=rrow, in_=stage)
                        else:
                            nc.vector.reciprocal(out=rrow, in_=pvt[64:65, :])
                        rbt = prec.tile([64, 512], F32, tag="rb")
                        nc.gpsimd.partition_broadcast(rbt, rrow)
                        nc.vector.tensor_tensor(
                            out=rt[hp][off : off + 64, sc * 512 : (sc + 1) * 512],
                            in0=pvt[0:64, :],
                            in1=rbt,
                            op=OP.mult,
                        )
                        if debug and b == 0 and h == 0:
                            nc.sync.dma_start(
                                out=dbg["dbg_rrow"][:, sc * 512 : (sc + 1) * 512],
                                in_=rrow,
                            )
                            nc.sync.dma_start(
                                out=dbg["dbg_rbt"][:, sc * 512 : (sc + 1) * 512],
                                in_=rbt,
                            )

                if debug and b == 0:
                    for hp in range(4):
                        nc.sync.dma_start(
                            out=dbg["dbg_rt"][hp * 128 : (hp + 1) * 128, :],
                            in_=rt[hp],
                        )
                # ---- output projection + bias + residual, DMA out
                for cb in range(4):
                    ot = pout.tile([128, S], F32, tag="ot")
                    for sc in range(2):
                        po = pps.tile([128, 512], F32, tag="pp")
                        for db in range(4):
                            nc.tensor.matmul(
                                out=po,
                                lhsT=wo_sb[db][:, cb * 128 : (cb + 1) * 128],
                                rhs=rt[db][:, sc * 512 : (sc + 1) * 512],
                                start=(db == 0),
                                stop=(db == 3),
                            )
                        dst_ap = ot[:, sc * 512 : (sc + 1) * 512]
                        xsrc = xt[cb][:, sc * 512 : (sc + 1) * 512]
                        if has_outb:
                            nc.vector.scalar_tensor_tensor(
                                out=dst_ap,
                                in0=po,
                                scalar=outb_sb[:, cb : cb + 1],
                                in1=xsrc,
                                op0=OP.add,
                                op1=OP.add,
                            )
                        else:
                            nc.vector.tensor_tensor(
                                out=dst_ap, in0=po, in1=xsrc, op=OP.add
                            )
                    nc.sync.dma_start(
                        out=out_d[b, cb * 128 : (cb + 1) * 128, :], in_=ot
                    )

    nc.finalize()
    return nc


def kernel(**inputs):
    x = np.asarray(inputs["x"], np.float32)
    norm_w = np.asarray(inputs["norm_w"], np.float64)
    norm_b = np.asarray(inputs["norm_b"], np.float64)
    proj_w = np.asarray(inputs["proj_w"], np.float64)
    proj_b = np.asarray(inputs["proj_b"], np.float64)
    out_w = np.asarray(inputs["out_w"], np.float32)
    out_b = np.asarray(inputs["out_b"], np.float32)

    # split qkv rows (row = h*192 + t*64 + d, t in {q,k,v}) into head-major mats
    pw = proj_w.reshape(NH, 3, DK, C)
    pb = proj_b.reshape(NH, 3, DK)
    mats, biases = [], []
    for t in range(3):
        wm = pw[:, t].reshape(NH * DK, C)
        bv = pb[:, t].reshape(NH * DK)
        # fold groupnorm affine: y = xhat*nw + nb  =>  W@y + b = (W*nw)@xhat + (W@nb + b)
        mats.append(wm * norm_w[None, :])
        biases.append(bv + wm @ norm_b)
    wq, wk, wv = mats
    bq, bk, bv = biases
    scale = DK ** -0.5
    wq = wq * scale
    bq = bq * scale

    wqT = np.ascontiguousarray(wq.T).astype(NPBF16)
    wkT = np.ascontiguousarray(wk.T).astype(NPBF16)
    wvT = np.ascontiguousarray(wv.T).astype(NPBF16)
    woT = np.ascontiguousarray(out_w.T).astype(NPBF16)

    G = np.zeros((128, 8), np.float32)
    G[np.arange(128), np.arange(128) // GSZ] = 1.0
    GT = np.ascontiguousarray(G.T)

    has_bqk = bool(np.any(bq) or np.any(bk))
    has_bv = bool(np.any(bv))
    has_outb = bool(np.any(out_b))

    bqk = np.zeros((128, 8), np.float32)
    bqk[:, 0:4] = bq.reshape(4, 128).T
    bqk[:, 4:8] = bk.reshape(4, 128).T
    outb128 = np.ascontiguousarray(out_b.reshape(4, 128).T)

    nc = _build(has_bqk, has_bv, has_outb)

    xr = x.reshape(B, C, S)
    in_maps = []
    for c in range(N_CORES):
        m = {
            "x": np.ascontiguousarray(xr[c * BL : (c + 1) * BL]),
            "wqt": wqT,
            "wkt": wkT,
            "wvt": wvT,
            "wot": woT,
            "gmat": G,
            "gtmat": GT,
        }
        if has_bqk:
            m["bqk"] = bqk
        if has_bv:
            m["bv"] = np.ascontiguousarray(bv.reshape(1, C)).astype(NPBF16)
        if has_outb:
            m["outb"] = outb128
        in_maps.append(m)

    res = run_bass_kernel_spmd(
        nc, in_maps, core_ids=list(range(N_CORES)), trace=TRACE
    )
    LAST["exec_time_ns"] = res.exec_time_ns
    LAST["mean_exec_time_ns"] = res.mean_exec_time_ns
    LAST["result"] = res

    out = np.concatenate([res.results[c]["out"] for c in range(N_CORES)], axis=0)
    return np.ascontiguousarray(out.reshape(B, C, H, W).astype(np.float32))


# revision 21
# speedup vs baseline: 1.6179x; 1.0565x over previous
"""Trainium2 Bass kernel for nn_AttentionBlock (GroupNorm + 8-head self-attention + residual).

Full inputs in, full output out. Sharding: data-parallel over batch across the
8 NeuronCores (16 batches -> 2 per core), weights replicated, no collectives.

Layout strategy (per core, per batch; C=512 channels, S=1024 tokens):
  - x and xhat live as [C, S] tiles (channels on partitions) so GroupNorm
    scale/bias are per-partition scalars.  Cross-partition group reductions
    (16 channels/group) and the broadcast back are tiny PE matmuls against
    one-hot group matrices.
  - Q^T, K^T computed as [qk_rows, S] (head-major rows); V as [S, 8*65] with a
    ones-column per head (row-sums of exp fall out of the P@V matmul).
  - scores are computed TRANSPOSED: scoresT[j, i] = k_j . q_i so that the
    softmax reduction (over j) aligns with the matmul contraction axis and no
    transposes are ever needed.  exp() runs on ScalarE straight out of PSUM.
  - P@V gives resU^T [65, S] (row 64 = softmax denominators); normalization is
    a reciprocal of the sums row + gpsimd partition_broadcast + one DVE mul.
  - out-projection consumes resT directly; residual-add fused in the epilogue.
Attention/projection matmuls run in bf16 (fp32 PSUM accumulation); groupnorm
statistics stay fp32.  The softmax max-subtraction is skipped: scores are
~N(0,1) by construction (standardized activations, 1/sqrt(dk) folded into the
Q weights host-side), so exp() stays comfortably in fp32 range.
"""

import numpy as np
import ml_dtypes

import concourse.bacc as bacc
import concourse.tile as tile
from concourse import mybir
from concourse.bass_utils import run_bass_kernel_spmd

N_CORES = 8
B, C, H, W = 16, 512, 32, 32
S = H * W                      # 1024
BL = B // N_CORES              # 2 batches per core
NH, DK = 8, 64
NG = 32                        # groupnorm groups
GSZ = C // NG                  # 16 channels per group
EPS = 1e-5
F32 = mybir.dt.float32
BF16 = mybir.dt.bfloat16
AF = mybir.ActivationFunctionType
OP = mybir.AluOpType
NPBF16 = ml_dtypes.bfloat16

# test.py can flip these; results stashed in LAST.
TRACE = False
RECIP_MODE = "approx_sbuf"  # "approx_sbuf" | "plain"
LAST = {}


def _build(has_bqk, has_bv, has_outb, debug=False):
    nc = bacc.Bacc()
    dbg = {}
    if debug:
        for nm, shp in (
            ("dbg_xh", [C, S]),
            ("dbg_qt", [C, S]),
            ("dbg_kt", [C, S]),
            ("dbg_v", [8, 128, NH * 65]),
            ("dbg_ex", [128, S]),
            ("dbg_pvt", [65, S]),
            ("dbg_rrow", [1, S]),
            ("dbg_rbt", [64, S]),
            ("dbg_rt", [C, S]),
        ):
            dbg[nm] = nc.dram_tensor(nm, shp, F32, kind="ExternalOutput")

    x_d = nc.dram_tensor("x", [BL, C, S], F32, kind="ExternalInput")
    wqt_d = nc.dram_tensor("wqt", [C, C], BF16, kind="ExternalInput")   # [c_in, q_row]
    wkt_d = nc.dram_tensor("wkt", [C, C], BF16, kind="ExternalInput")
    wvt_d = nc.dram_tensor("wvt", [C, C], BF16, kind="ExternalInput")
    wot_d = nc.dram_tensor("wot", [C, C], BF16, kind="ExternalInput")   # [d_out, c_out]
    g_d = nc.dram_tensor("gmat", [128, 8], F32, kind="ExternalInput")
    gt_d = nc.dram_tensor("gtmat", [8, 128], F32, kind="ExternalInput")
    bqk_d = (
        nc.dram_tensor("bqk", [128, 8], F32, kind="ExternalInput") if has_bqk else None
    )
    bv_d = nc.dram_tensor("bv", [1, C], BF16, kind="ExternalInput") if has_bv else None
    outb_d = (
        nc.dram_tensor("outb", [128, 4], F32, kind="ExternalInput") if has_outb else None
    )
    out_d = nc.dram_tensor("out", [BL, C, S], F32, kind="ExternalOutput")

    with tile.TileContext(nc) as tc:
        with (
            tc.tile_pool(name="const", bufs=1) as const,
            tc.tile_pool(name="px", bufs=8) as px,
            tc.tile_pool(name="pgn", bufs=4) as pgn,
            tc.tile_pool(name="pxh", bufs=8) as pxh,
            tc.tile_pool(name="pqt", bufs=8) as pqt,
            tc.tile_pool(name="pkt", bufs=8) as pkt,
            tc.tile_pool(name="pv", bufs=16) as pvp,
            tc.tile_pool(name="pexp", bufs=10) as pexp,
            tc.tile_pool(name="prec", bufs=3) as prec,
            tc.tile_pool(name="prt", bufs=8) as prt,
            tc.tile_pool(name="pout", bufs=3) as pout,
            tc.tile_pool(name="pps", bufs=2, space="PSUM") as pps,
            tc.tile_pool(name="psc", bufs=2, space="PSUM") as psc,
            tc.tile_pool(name="ppv", bufs=2, space="PSUM") as ppv,
        ):
            # ---- constants into SBUF
            wq_sb, wk_sb, wv_sb, wo_sb = [], [], [], []
            for nm, lst, src in (
                ("q", wq_sb, wqt_d),
                ("k", wk_sb, wkt_d),
                ("v", wv_sb, wvt_d),
                ("o", wo_sb, wot_d),
            ):
                for cb in range(4):
                    t = const.tile([128, C], BF16, tag=f"w_{nm}_{cb}")
                    nc.sync.dma_start(out=t, in_=src[cb * 128 : (cb + 1) * 128, :])
                    lst.append(t)
            g_sb = const.tile([128, 8], F32, tag="g")
            nc.sync.dma_start(out=g_sb, in_=g_d[:, :])
            gt_sb = const.tile([8, 128], F32, tag="gt")
            nc.sync.dma_start(out=gt_sb, in_=gt_d[:, :])
            eps_sb = const.tile([128, 1], F32, tag="eps")
            nc.vector.memset(eps_sb, EPS)
            if has_bqk:
                bqk_sb = const.tile([128, 8], F32, tag="bqk")
                nc.sync.dma_start(out=bqk_sb, in_=bqk_d[:, :])
            if has_bv:
                bv_sb = const.tile([1, C], BF16, tag="bv")
                nc.sync.dma_start(out=bv_sb, in_=bv_d[:, :])
                ones_sb = const.tile([1, S], BF16, tag="ones")
                nc.vector.memset(ones_sb, 1.0)
            if has_outb:
                outb_sb = const.tile([128, 4], F32, tag="outb")
                nc.sync.dma_start(out=outb_sb, in_=outb_d[:, :])

            # ================= emission helpers =================
            def load_x(b):
                xt = []
                for cb in range(4):
                    t = px.tile([128, S], F32, tag="x", name=f"x{b}_{cb}")
                    nc.sync.dma_start(out=t, in_=x_d[b, cb * 128 : (cb + 1) * 128, :])
                    xt.append(t)
                return xt

            def gn_block(b, xt, xh, cb):
                # groupnorm -> xhat (standardized; w/b folded into weights)
                st6 = pgn.tile([128, 2, 6], F32, tag="st6")
                nc.vector.bn_stats(out=st6[:, 0, :], in_=xt[cb][:, 0:512])
                nc.vector.bn_stats(out=st6[:, 1, :], in_=xt[cb][:, 512:1024])
                mv = pgn.tile([128, 2], F32, tag="mv")
                nc.vector.bn_aggr(out=mv, in_=st6)
                # me2 = [mean_c, E[x^2]_c]
                me2 = pgn.tile([128, 2], F32, tag="me2")
                nc.vector.tensor_copy(out=me2[:, 0:1], in_=mv[:, 0:1])
                nc.vector.tensor_tensor(
                    out=me2[:, 1:2], in0=mv[:, 0:1], in1=mv[:, 0:1], op=OP.mult
                )
                nc.vector.tensor_tensor(
                    out=me2[:, 1:2], in0=me2[:, 1:2], in1=mv[:, 1:2], op=OP.add
                )
                # aggregate the 8 groups in this channel block (16 ch each)
                pg = pps.tile([8, 2], F32, tag="pp")
                nc.tensor.matmul(out=pg, lhsT=g_sb, rhs=me2, start=True, stop=True)
                gs = pgn.tile([8, 2], F32, tag="gs")  # [gmean, grstd]
                nc.scalar.mul(out=gs[:, 0:1], in_=pg[:, 0:1], mul=1.0 / GSZ)
                tmp = pgn.tile([8, 2], F32, tag="tmp")
                nc.vector.tensor_tensor(
                    out=tmp[:, 0:1], in0=gs[:, 0:1], in1=gs[:, 0:1], op=OP.mult
                )
                nc.vector.tensor_scalar(
                    out=tmp[:, 1:2],
                    in0=pg[:, 1:2],
                    scalar1=1.0 / GSZ,
                    scalar2=None,
                    op0=OP.mult,
                )
                nc.vector.tensor_tensor(
                    out=tmp[:, 1:2], in0=tmp[:, 1:2], in1=tmp[:, 0:1], op=OP.subtract
                )
                nc.scalar.activation(
                    out=gs[:, 1:2], in_=tmp[:, 1:2], func=AF.Sqrt, bias=eps_sb[0:8, :]
                )
                nc.vector.reciprocal(out=gs[:, 1:2], in_=gs[:, 1:2])
                # broadcast group stats back to the 128 channels
                pb = pps.tile([128, 2], F32, tag="pp")
                nc.tensor.matmul(out=pb, lhsT=gt_sb, rhs=gs, start=True, stop=True)
                t = pxh.tile([128, S], BF16, tag="xh", name=f"xh{b}_{cb}")
                nc.vector.tensor_scalar(
                    out=t,
                    in0=xt[cb],
                    scalar1=pb[:, 0:1],
                    scalar2=pb[:, 1:2],
                    op0=OP.subtract,
                    op1=OP.mult,
                )
                xh.append(t)
                if debug and b == 0:
                    nc.sync.dma_start(
                        out=dbg["dbg_xh"][cb * 128 : (cb + 1) * 128, :],
                        in_=t,
                    )

            def qk_group(b, xh, dst, w_sb, boff, rb):
                # one [128, S] row-block of Q^T or K^T
                pool = pqt if boff == 0 else pkt
                t = pool.tile([128, S], BF16, tag="qk", name=f"qk{b}_{boff}_{rb}")
                for sc in range(2):
                    pq = pps.tile([128, 512], F32, tag="pp")
                    for cb in range(4):
                        nc.tensor.matmul(
                            out=pq,
                            lhsT=w_sb[cb][:, rb * 128 : (rb + 1) * 128],
                            rhs=xh[cb][:, sc * 512 : (sc + 1) * 512],
                            start=(cb == 0),
                            stop=(cb == 3),
                        )
                    dst_ap = t[:, sc * 512 : (sc + 1) * 512]
                    if has_bqk:
                        nc.vector.tensor_scalar_add(
                            out=dst_ap,
                            in0=pq,
                            scalar1=bqk_sb[:, boff + rb : boff + rb + 1],
                        )
                    else:
                        nc.vector.tensor_copy(out=dst_ap, in_=pq)
                dst.append(t)
                if debug and b == 0:
                    dnm = "dbg_qt" if boff == 0 else "dbg_kt"
                    nc.sync.dma_start(
                        out=dbg[dnm][rb * 128 : (rb + 1) * 128, :], in_=t
                    )

            def v_group(b, xh, vt, st):
                # one [S-tile, NH, 65] V tile with ones column per head
                pv = pps.tile([128, 512], F32, tag="pp")
                for cb in range(4):
                    nc.tensor.matmul(
                        out=pv,
                        lhsT=xh[cb][:, st * 128 : (st + 1) * 128],
                        rhs=wv_sb[cb],
                        start=(cb == 0),
                        stop=(cb == 3 and not has_bv),
                    )
                if has_bv:
                    nc.tensor.matmul(
                        out=pv,
                        lhsT=ones_sb[:, st * 128 : (st + 1) * 128],
                        rhs=bv_sb,
                        start=False,
                        stop=True,
                    )
                t = pvp.tile([128, NH, 65], BF16, tag="v", name=f"v{b}_{st}")
                nc.vector.memset(t[:, :, 64:65], 1.0)
                nc.vector.tensor_copy(
                    out=t[:, :, 0:64], in_=pv.rearrange("p (h d) -> p h d", h=NH)
                )
                vt.append(t)
                if debug and b == 0:
                    nc.sync.dma_start(
                        out=dbg["dbg_v"][st, :, :],
                        in_=t.rearrange("p h d -> p (h d)"),
                    )

            def attn_head(b, qt, kt, vt, rt, h):
                # scores transposed -> exp -> P@V halves -> normalized resT rows
                hp, off = h // 2, (h % 2) * 64
                ex_tiles = [None] * 8
                for jb in range(8):
                    ps = psc.tile([128, S], F32, tag="ps")
                    for sc in range(2):
                        nc.tensor.matmul(
                            out=ps[:, sc * 512 : (sc + 1) * 512],
                            lhsT=kt[hp][off : off + 64, jb * 128 : (jb + 1) * 128],
                            rhs=qt[hp][off : off + 64, sc * 512 : (sc + 1) * 512],
                            start=True,
                            stop=True,
                        )
                    ex = pexp.tile([128, S], BF16, tag="ex")
                    nc.scalar.activation(out=ex, in_=ps, func=AF.Exp)
                    ex_tiles[jb] = ex
                    if debug and b == 0 and h == 0 and jb == 0:
                        nc.sync.dma_start(out=dbg["dbg_ex"][:, :], in_=ex)

                # P@V per 512-column half into a single-bank accumulator;
                # each half's normalize overlaps the other half's matmuls.
                for sc in range(2):
                    pvt = ppv.tile([65, 512], F32, tag="ppvt")
                    for jb in range(8):
                        nc.tensor.matmul(
                            out=pvt,
                            lhsT=vt[jb][:, h, :],
                            rhs=ex_tiles[jb][:, sc * 512 : (sc + 1) * 512],
                            start=(jb == 0),
                            stop=(jb == 7),
                        )
                    if debug and b == 0 and h == 0:
                        dump = pout.tile([65, 512], F32, tag="dump", name=f"dmp{sc}")
                        nc.vector.tensor_copy(out=dump, in_=pvt)
                        nc.sync.dma_start(
                            out=dbg["dbg_pvt"][:, sc * 512 : (sc + 1) * 512],
                            in_=dump,
                        )
                    # normalize: resT[head rows] = resU / rowsum
                    rrow = prec.tile([1, 512], F32, tag="rr")
                    if RECIP_MODE == "approx_sbuf":
                        stage = prec.tile([1, 512], F32, tag="st")
                        nc.vector.tensor_copy(out=stage, in_=pvt[64:65, :])
                        nc.vector.reciprocal_approx_fast(out=rrow, in_=stage)
                    else:
                        nc.vector.reciprocal(out=rrow, in_=pvt[64:65, :])
                    rbt = prec.tile([64, 512], F32, tag="rb")
                    nc.gpsimd.partition_broadcast(rbt, rrow)
                    nc.vector.tensor_tensor(
                        out=rt[hp][off : off + 64, sc * 512 : (sc + 1) * 512],
                        in0=pvt[0:64, :],
                        in1=rbt,
                        op=OP.mult,
                    )
                    if debug and b == 0 and h == 0:
                        nc.sync.dma_start(
                            out=dbg["dbg_rrow"][:, sc * 512 : (sc + 1) * 512],
                            in_=rrow,
                        )
                        nc.sync.dma_start(
                            out=dbg["dbg_rbt"][:, sc * 512 : (sc + 1) * 512],
                            in_=rbt,
                        )

            def epi_block(b, rt, xt, cb):
                # output projection + bias + residual for one channel block
                ot = pout.tile([128, S], F32, tag="ot")
                for sc in range(2):
                    po = pps.tile([128, 512], F32, tag="pp")
                    for db in range(4):
                        nc.tensor.matmul(
                            out=po,
                            lhsT=wo_sb[db][:, cb * 128 : (cb + 1) * 128],
                            rhs=rt[db][:, sc * 512 : (sc + 1) * 512],
                            start=(db == 0),
                            stop=(db == 3),
                        )
                    dst_ap = ot[:, sc * 512 : (sc + 1) * 512]
                    xsrc = xt[cb][:, sc * 512 : (sc + 1) * 512]
                    if has_outb:
                        nc.vector.scalar_tensor_tensor(
                            out=dst_ap,
                            in0=po,
                            scalar=outb_sb[:, cb : cb + 1],
                            in1=xsrc,
                            op0=OP.add,
                            op1=OP.add,
                        )
                    else:
                        nc.vector.tensor_tensor(out=dst_ap, in0=po, in1=xsrc, op=OP.add)
                nc.sync.dma_start(out=out_d[b, cb * 128 : (cb + 1) * 128, :], in_=ot)

            def drain(wl, n):
                for _ in range(min(n, len(wl))):
                    wl.pop(0)()

            # ================= schedule =================
            # batch 0 prep emitted directly; batch 1 prep + batch 0 epilogue are
            # emitted interleaved into the attention head loops so the Tensor
            # engine always has dense independent work while ScalarE exps.
            xt0 = load_x(0)
            xh0, qt0, kt0, vt0 = [], [], [], []
            for cb in range(4):
                gn_block(0, xt0, xh0, cb)
            for rb in range(4):
                qk_group(0, xh0, qt0, wq_sb, 0, rb)
                qk_group(0, xh0, kt0, wk_sb, 4, rb)
            for st in range(8):
                v_group(0, xh0, vt0, st)

            xt1 = load_x(1)
            xh1, qt1, kt1, vt1 = [], [], [], []
            work1 = []
            for cb in range(4):
                work1.append(lambda cb=cb: gn_block(1, xt1, xh1, cb))
            for rb in range(4):
                work1.append(lambda rb=rb: qk_group(1, xh1, qt1, wq_sb, 0, rb))
                work1.append(lambda rb=rb: qk_group(1, xh1, kt1, wk_sb, 4, rb))
            for st in range(8):
                work1.append(lambda st=st: v_group(1, xh1, vt1, st))

            rt0 = [prt.tile([128, S], BF16, tag="rt", name=f"rt0_{i}") for i in range(4)]
            for h in range(NH):
                attn_head(0, qt0, kt0, vt0, rt0, h)
                drain(work1, 4)
            drain(work1, len(work1))
            if debug:
                for hp in range(4):
                    nc.sync.dma_start(
                        out=dbg["dbg_rt"][hp * 128 : (hp + 1) * 128, :], in_=rt0[hp]
                    )

            epi0 = [lambda cb=cb: epi_block(0, rt0, xt0, cb) for cb in range(4)]
            rt1 = [prt.tile([128, S], BF16, tag="rt", name=f"rt1_{i}") for i in range(4)]
            for h in range(NH):
                attn_head(1, qt1, kt1, vt1, rt1, h)
                drain(epi0, 1)
            drain(epi0, len(epi0))
            for cb in range(4):
                epi_block(1, rt1, xt1, cb)

    nc.finalize()
    return nc


def kernel(**inputs):
    x = np.asarray(inputs["x"], np.float32)
    norm_w = np.asarray(inputs["norm_w"], np.float64)
    norm_b = np.asarray(inputs["norm_b"], np.float64)
    proj_w = np.asarray(inputs["proj_w"], np.float64)
    proj_b = np.asarray(inputs["proj_b"], np.float64)
    out_w = np.asarray(inputs["out_w"], np.float32)
    out_b = np.asarray(inputs["out_b"], np.float32)

    # split qkv rows (row = h*192 + t*64 + d, t in {q,k,v}) into head-major mats
    pw = proj_w.reshape(NH, 3, DK, C)
    pb = proj_b.reshape(NH, 3, DK)
    mats, biases = [], []
    for t in range(3):
        wm = pw[:, t].reshape(NH * DK, C)
        bv = pb[:, t].reshape(NH * DK)
        # fold groupnorm affine: y = xhat*nw + nb  =>  W@y + b = (W*nw)@xhat + (W@nb + b)
        mats.append(wm * norm_w[None, :])
        biases.append(bv + wm @ norm_b)
    wq, wk, wv = mats
    bq, bk, bv = biases
    scale = DK ** -0.5
    wq = wq * scale
    bq = bq * scale

    wqT = np.ascontiguousarray(wq.T).astype(NPBF16)
    wkT = np.ascontiguousarray(wk.T).astype(NPBF16)
    wvT = np.ascontiguousarray(wv.T).astype(NPBF16)
    woT = np.ascontiguousarray(out_w.T).astype(NPBF16)

    G = np.zeros((128, 8), np.float32)
    G[np.arange(128), np.arange(128) // GSZ] = 1.0
    GT = np.ascontiguousarray(G.T)

    has_bqk = bool(np.any(bq) or np.any(bk))
    has_bv = bool(np.any(bv))
    has_outb = bool(np.any(out_b))

    bqk = np.zeros((128, 8), np.float32)
    bqk[:, 0:4] = bq.reshape(4, 128).T
    bqk[:, 4:8] = bk.reshape(4, 128).T
    outb128 = np.ascontiguousarray(out_b.reshape(4, 128).T)

    nc = _build(has_bqk, has_bv, has_outb)

    xr = x.reshape(B, C, S)
    in_maps = []
    for c in range(N_CORES):
        m = {
            "x": np.ascontiguousarray(xr[c * BL : (c + 1) * BL]),
            "wqt": wqT,
            "wkt": wkT,
            "wvt": wvT,
            "wot": woT,
            "gmat": G,
            "gtmat": GT,
        }
        if has_bqk:
            m["bqk"] = bqk
        if has_bv:
            m["bv"] = np.ascontiguousarray(bv.reshape(1, C)).astype(NPBF16)
        if has_outb:
            m["outb"] = outb128
        in_maps.append(m)

    res = run_bass_kernel_spmd(
        nc, in_maps, core_ids=list(range(N_CORES)), trace=TRACE
    )
    LAST["exec_time_ns"] = res.exec_time_ns
    LAST["mean_exec_time_ns"] = res.mean_exec_time_ns
    LAST["result"] = res

    out = np.concatenate([res.results[c]["out"] for c in range(N_CORES)], axis=0)
    return np.ascontiguousarray(out.reshape(B, C, H, W).astype(np.float32))
